# revision 11
# baseline (speedup 1.0000x reference)
"""Trainium2 Bass kernel for nn_ByteSequenceEmbedder.

Data-parallel across 8 NeuronCores: 2 sequences per core, weights replicated.

Per-core dataflow (all activations channels-on-partitions, "layout A" [C, T]):
  embed   : one-hot matmul — tokens broadcast [128,T] (host), DVE is_equal vs
            per-partition iota -> onehot chunks; PE: tok_emb-chunks.T @ onehot
            accumulated in PSUM (+ K=1 matmul adding bpe-marker row)
  conv0   : 3 shifted matmuls per (T-chunk, co-chunk) accumulating in PSUM,
            ReLU+bias fused into the ACT PSUM->SBUF evacuation
  highway : 2 blocks x 2 layers; 8x4 matmuls per T-chunk, ReLU/Sigmoid evac,
            DVE combine x' = g*(relu(h)-x)+x
  conv1   : 12 matmuls per (T-chunk, co-chunk) + residual add
  pool    : ragged word max-pool as masked shifted max:
            msel[t] = max(x2[t], x2[t+1]+A1[t], x2[t+2]+A2[t]) with host-built
            additive masks (0 where word@t has len>j, else -1e30)
  proj    : projection applied over ALL T positions; host selects column s_w
            per word while unsharding (empty pools -> proj_b row)

Matmul operands are bf16 (f32 PSUM accumulation).
"""
import numpy as np

import concourse.bacc as bacc
import concourse.tile as tile
import concourse.mybir as mybir

BSZ, NW, T = 16, 1024, 3072
BED, WED = 128, 512
VOCAB = 264
BPE_MASK_IDX = 4
N_CORES = 8
SEQ_PER_CORE = BSZ // N_CORES
TP = T + 2          # conv buffers: one zero halo col each side
TP2 = T + 4         # pooling source: 1 left + 3 right halo cols
NCH = T // 512      # T-chunks of 512
BF16 = mybir.dt.bfloat16
F16 = mybir.dt.float16
F32 = mybir.dt.float32

_BF16_NP = mybir.dt.np(BF16)
_F16_NP = np.float16
NEG_BIG = -1e30

_CACHE = {}


def _build_program():
    nc = bacc.Bacc("TRN2", target_bir_lowering=False, debug=False)

    def dram_in(name, shape, dt):
        return nc.dram_tensor(name, shape, dt, kind="ExternalInput").ap()

    emb_lhs = dram_in("emb_lhs", [128, 3 * 128], BF16)   # tok_emb row-chunks
    emb_row4 = dram_in("emb_row4", [1, 128], BF16)       # tok_emb[4]
    iota_c = dram_in("iota_c", [128, 3], F32)            # per-partition vocab iota
    w_c0 = dram_in("w_c0", [128, 3 * WED], BF16)         # [ci, k*512+co]
    w_c1 = dram_in("w_c1", [128, 4 * 3 * WED], BF16)     # [ci%128, (q*3+k)*512+co]
    w_hw = dram_in("w_hw", [128, 4 * 4 * 1024], BF16)    # [(bl*4+q)*1024 + co_out]
    w_pr = dram_in("w_pr", [128, 4 * WED], BF16)         # [q*512+co]
    b_c0 = dram_in("b_c0", [128, 4], F32)
    b_c1 = dram_in("b_c1", [128, 4], F32)
    b_hw = dram_in("b_hw", [128, 4 * 8], F32)            # [bl*8 + m]
    b_pr = dram_in("b_pr", [128, 4], F32)
    tok_bc = dram_in("tok_bc", [SEQ_PER_CORE, 128, T], F16)   # tokens bcast over partitions
    bpe_row = dram_in("bpe_row", [SEQ_PER_CORE, 1, T], BF16)  # bpe mask 0/1
    a_msk = dram_in("a_msk", [SEQ_PER_CORE, 128, 2 * T], BF16)  # pooling additive masks

    out = nc.dram_tensor("out", [SEQ_PER_CORE, WED, T], F32, kind="ExternalOutput").ap()

    RELU = mybir.ActivationFunctionType.Relu
    SIGM = mybir.ActivationFunctionType.Sigmoid
    IDEN = mybir.ActivationFunctionType.Identity
    MAX = mybir.AluOpType.max
    ADD = mybir.AluOpType.add
    SUB = mybir.AluOpType.subtract
    MUL = mybir.AluOpType.mult
    ISEQ = mybir.AluOpType.is_equal

    with tile.TileContext(nc) as tc:
        with tc.tile_pool(name="wp", bufs=1) as wp, \
             tc.tile_pool(name="ap", bufs=1) as apool, \
             tc.tile_pool(name="tp", bufs=3) as tp, \
             tc.tile_pool(name="pp", bufs=8, space="PSUM") as pp:

            # ---- load weights/biases once (sync queue; small/early first) ----
            t_embA = wp.tile([128, 3 * 128], BF16)
            t_row4 = wp.tile([1, 128], BF16)
            t_iota = wp.tile([128, 3], F32)
            t_bc0 = wp.tile([128, 4], F32)
            t_bc1 = wp.tile([128, 4], F32)
            t_bhw = wp.tile([128, 4 * 8], F32)
            t_bpr = wp.tile([128, 4], F32)
            t_wc0 = wp.tile([128, 3 * WED], BF16)
            t_wc1 = wp.tile([128, 4 * 3 * WED], BF16)
            t_whw = wp.tile([128, 4 * 4 * 1024], BF16)
            t_wpr = wp.tile([128, 4 * WED], BF16)
            for t, d in ((t_embA, emb_lhs), (t_row4, emb_row4), (t_iota, iota_c),
                         (t_bc0, b_c0), (t_bc1, b_c1), (t_bhw, b_hw), (t_bpr, b_pr),
                         (t_wc0, w_c0), (t_wc1, w_c1), (t_whw, w_hw), (t_wpr, w_pr)):
                nc.sync.dma_start(out=t[:], in_=d[:])

            def conv_block(X, Y, wt, bt, n_ci):
                """Y[:, chunk m cols 1..T] = relu(conv(X) + b)."""
                for n in range(NCH):
                    for m in range(4):
                        ps = pp.tile([128, 512], F32, tag="ps", name="ps")
                        nmm = n_ci * 3
                        i = 0
                        for q in range(n_ci):
                            for k in range(3):
                                lhs = wt[:, (q * 3 + k) * WED + m * 128:(q * 3 + k) * WED + (m + 1) * 128]
                                nc.tensor.matmul(
                                    out=ps[:], lhsT=lhs,
                                    rhs=X[:, q * TP + n * 512 + k:q * TP + n * 512 + k + 512],
                                    start=(i == 0), stop=(i == nmm - 1))
                                i += 1
                        dst = Y[:, m * TP + 1 + n * 512:m * TP + 1 + (n + 1) * 512]
                        nc.scalar.activation(out=dst, in_=ps[:], func=RELU,
                                             bias=bt[:, m:m + 1], scale=1.0)

            def highway_layer(X, Y, bl, ytp=TP):
                """Y = g*relu(h) + (1-g)*X; X [128, 4*TP], Y [128, 4*ytp]."""
                for n in range(NCH):
                    pss = []
                    for m in range(8):
                        ps = pp.tile([128, 512], F32, tag="ps", name="ps")
                        for q in range(4):
                            base = (bl * 4 + q) * 1024 + m * 128
                            nc.tensor.matmul(
                                out=ps[:], lhsT=t_whw[:, base:base + 128],
                                rhs=X[:, q * TP + 1 + n * 512:q * TP + 1 + (n + 1) * 512],
                                start=(q == 0), stop=(q == 3))
                        pss.append(ps)
                    for c in range(4):
                        xs = X[:, c * TP + 1 + n * 512:c * TP + 1 + (n + 1) * 512]
                        h_t = tp.tile([128, 512], BF16, tag="h", name="h_t")
                        g_t = tp.tile([128, 512], BF16, tag="g", name="g_t")
                        d_t = tp.tile([128, 512], BF16, tag="d", name="d_t")
                        nc.scalar.activation(out=h_t[:], in_=pss[c][:], func=RELU,
                                             bias=t_bhw[:, bl * 8 + c:bl * 8 + c + 1], scale=1.0)
                        nc.scalar.activation(out=g_t[:], in_=pss[4 + c][:], func=SIGM,
                                             bias=t_bhw[:, bl * 8 + 4 + c:bl * 8 + 4 + c + 1], scale=1.0)
                        nc.vector.tensor_tensor(out=d_t[:], in0=h_t[:], in1=xs, op=SUB)
                        nc.vector.tensor_tensor(out=d_t[:], in0=d_t[:], in1=g_t[:], op=MUL)
                        ys = Y[:, c * ytp + 1 + n * 512:c * ytp + 1 + (n + 1) * 512]
                        nc.vector.tensor_tensor(out=ys, in0=d_t[:], in1=xs, op=ADD)

            scope = nc.named_scope

            def embed_seq(s):
                """One-hot-matmul embedding for sequence s -> x0 [128, TP] bf16.
                Token DMA is chunked so the first compare starts early."""
                ctx = scope(f"s{s}_embed"); ctx.__enter__()
                t_tok = apool.tile([128, T], F16, tag="tok", name="t_tok", bufs=2)
                t_bpe = apool.tile([1, T], BF16, tag="bpe", name="t_bpe", bufs=2)
                t_am = apool.tile([128, 2 * T], BF16, tag="am", name="t_am")
                for n in range(NCH):
                    nc.scalar.dma_start(out=t_tok[:, n * 512:(n + 1) * 512],
                                        in_=tok_bc[s, :, n * 512:(n + 1) * 512])
                nc.scalar.dma_start(out=t_bpe[:], in_=bpe_row[s])
                nc.scalar.dma_start(out=t_am[:], in_=a_msk[s])

                x0 = apool.tile([128, TP], BF16, tag="x0", name="x0", bufs=2)
                nc.vector.memset(x0[:, 0:1], 0)
                nc.vector.memset(x0[:, TP - 1:TP], 0)
                for n in range(NCH):
                    oh1 = tp.tile([128, 512], BF16, tag="oh1", name="oh1")
                    oh2 = tp.tile([128, 512], BF16, tag="oh2", name="oh2")
                    oh3 = tp.tile([8, 512], BF16, tag="oh3", name="oh3")
                    tb = t_tok[:, n * 512:(n + 1) * 512]
                    nc.vector.tensor_scalar(out=oh1[:], in0=tb, scalar1=t_iota[:, 0:1],
                                            scalar2=None, op0=ISEQ)
                    nc.vector.tensor_scalar(out=oh2[:], in0=tb, scalar1=t_iota[:, 1:2],
                                            scalar2=None, op0=ISEQ)
                    nc.vector.tensor_scalar(out=oh3[:], in0=t_tok[0:8, n * 512:(n + 1) * 512],
                                            scalar1=t_iota[0:8, 2:3], scalar2=None, op0=ISEQ)
                    ps = pp.tile([128, 512], F32, tag="ps", name="ps")
                    nc.tensor.matmul(out=ps[:], lhsT=t_embA[:, 0:128], rhs=oh1[:],
                                     start=True, stop=False)
                    nc.tensor.matmul(out=ps[:], lhsT=t_embA[:, 128:256], rhs=oh2[:],
                                     start=False, stop=False)
                    nc.tensor.matmul(out=ps[:], lhsT=t_embA[0:8, 256:384], rhs=oh3[:],
                                     start=False, stop=False)
                    nc.tensor.matmul(out=ps[:], lhsT=t_row4[:], rhs=t_bpe[:, n * 512:(n + 1) * 512],
                                     start=False, stop=True)
                    nc.scalar.activation(out=x0[:, 1 + n * 512:1 + (n + 1) * 512],
                                         in_=ps[:], func=IDEN, bias=0.0, scale=1.0)
                ctx.__exit__(None, None, None)
                return x0, t_am

            x0, t_am = embed_seq(0)
            for s in range(SEQ_PER_CORE):

                def act_buf(tag, w=TP, extra_halo=0):
                    b = apool.tile([128, 4 * w], BF16, tag=tag, name=tag)
                    for q in range(4):
                        nc.vector.memset(b[:, q * w:q * w + 1], 0)
                        nc.vector.memset(b[:, q * w + 1 + T:(q + 1) * w], 0)
                    return b

                # ---------- conv0 + highway block 0 ----------
                with scope(f"s{s}_conv0"):
                    x1 = act_buf("actA")
                    conv_block(x0, x1, t_wc0, t_bc0, 1)
                with scope(f"s{s}_hw0l0"):
                    x1b = act_buf("actB")
                    highway_layer(x1, x1b, 0)
                with scope(f"s{s}_hw0l1"):
                    x1c = act_buf("actC")
                    highway_layer(x1b, x1c, 1)

                # ---------- conv1 (+res) + highway block 1 ----------
                ctx = scope(f"s{s}_conv1"); ctx.__enter__()
                x2p = act_buf("actA")
                for n in range(NCH):
                    for m in range(4):
                        ps = pp.tile([128, 512], F32, tag="ps", name="ps")
                        i = 0
                        for q in range(4):
                            for k in range(3):
                                lhs = t_wc1[:, (q * 3 + k) * WED + m * 128:(q * 3 + k) * WED + (m + 1) * 128]
                                nc.tensor.matmul(
                                    out=ps[:], lhsT=lhs,
                                    rhs=x1c[:, q * TP + n * 512 + k:q * TP + n * 512 + k + 512],
                                    start=(i == 0), stop=(i == 11))
                                i += 1
                        r_t = tp.tile([128, 512], BF16, tag="h", name="r_t")
                        nc.scalar.activation(out=r_t[:], in_=ps[:], func=RELU,
                                             bias=t_bc1[:, m:m + 1], scale=1.0)
                        xs = x1c[:, m * TP + 1 + n * 512:m * TP + 1 + (n + 1) * 512]
                        nc.vector.tensor_tensor(
                            out=x2p[:, m * TP + 1 + n * 512:m * TP + 1 + (n + 1) * 512],
                            in0=r_t[:], in1=xs, op=ADD)
                ctx.__exit__(None, None, None)

                with scope(f"s{s}_hw1l0"):
                    x2b = act_buf("actB")
                    highway_layer(x2p, x2b, 2)
                with scope(f"s{s}_hw1l1"):
                    x2 = act_buf("actC", w=TP2)
                    highway_layer(x2b, x2, 3, ytp=TP2)

                # prefetch next sequence's embedding while PE is light
                if s + 1 < SEQ_PER_CORE:
                    next_x0, next_am = embed_seq(s + 1)

                # ---------- ragged max pool + projection, pipelined per T-chunk ----------
                ctx = scope(f"s{s}_poolproj"); ctx.__enter__()
                msel = apool.tile([128, 4 * T], BF16, tag="actB", name="msel")
                for n in range(NCH):
                    lo, hi = n * 512, (n + 1) * 512
                    for c in range(4):
                        base = c * TP2 + 1
                        s1 = tp.tile([128, 512], BF16, tag="s1", name="s1")
                        s2 = tp.tile([128, 512], BF16, tag="s2", name="s2")
                        nc.vector.tensor_tensor(out=s1[:], in0=x2[:, base + 1 + lo:base + 1 + hi],
                                                in1=t_am[:, lo:hi], op=ADD)
                        nc.vector.tensor_tensor(out=s2[:], in0=x2[:, base + 2 + lo:base + 2 + hi],
                                                in1=t_am[:, T + lo:T + hi], op=ADD)
                        nc.vector.tensor_tensor(out=s1[:], in0=s1[:], in1=s2[:], op=MAX)
                        nc.vector.tensor_tensor(out=msel[:, c * T + lo:c * T + hi],
                                                in0=s1[:], in1=x2[:, base + lo:base + hi], op=MAX)
                    for m in range(4):
                        ps = pp.tile([128, 512], F32, tag="ps", name="ps")
                        for q in range(4):
                            nc.tensor.matmul(
                                out=ps[:], lhsT=t_wpr[:, q * WED + m * 128:q * WED + (m + 1) * 128],
                                rhs=msel[:, q * T + lo:q * T + hi],
                                start=(q == 0), stop=(q == 3))
                        o_t = tp.tile([128, 512], F32, tag="o", name="o_t", bufs=4)
                        nc.scalar.activation(out=o_t[:], in_=ps[:], func=IDEN,
                                             bias=t_bpr[:, m:m + 1], scale=1.0)
                        nc.sync.dma_start(out=out[s, m * 128:(m + 1) * 128, lo:hi], in_=o_t[:])
                ctx.__exit__(None, None, None)
                if s + 1 < SEQ_PER_CORE:
                    x0, t_am = next_x0, next_am

    nc.compile()
    return nc


def _prep_inputs(inputs):
    """Host-side: shard + convert to the kernel's DRAM tensor layouts."""
    byte_tokens = np.asarray(inputs["byte_tokens"], np.int64)
    bpe_mask = np.asarray(inputs["bpe_mask"], bool)
    pool_lengths = np.asarray(inputs["pool_lengths"], np.int64)
    tok_emb = np.asarray(inputs["tok_emb"], np.float32)

    def bf(x):
        return np.ascontiguousarray(np.asarray(x, np.float32).astype(_BF16_NP))

    conv0_W = np.asarray(inputs["conv0_W"], np.float32)   # [3,128,512]
    conv1_W = np.asarray(inputs["conv1_W"], np.float32)   # [3,512,512]
    hw0_W = np.asarray(inputs["hw0_W"], np.float32)       # [2,1024,512]
    hw1_W = np.asarray(inputs["hw1_W"], np.float32)
    proj_W = np.asarray(inputs["proj_W"], np.float32)     # [512,512]

    w_c0 = bf(conv0_W.transpose(1, 0, 2).reshape(128, 3 * WED))
    w_c1 = bf(conv1_W.transpose(1, 0, 2).reshape(4, 128, 3, WED)
              .transpose(1, 0, 2, 3).reshape(128, 4 * 3 * WED))
    whw = np.empty((128, 16, 1024), np.float32)
    for bl, (blk, lay) in enumerate(((hw0_W, 0), (hw0_W, 1), (hw1_W, 0), (hw1_W, 1))):
        wt = blk[lay].T  # [512, 1024]
        for q in range(4):
            whw[:, bl * 4 + q, :] = wt[q * 128:(q + 1) * 128]
    w_hw = bf(whw.reshape(128, 16 * 1024))
    w_pr = bf(proj_W.T.reshape(4, 128, WED).transpose(1, 0, 2).reshape(128, 4 * WED))

    def colchunks(b):  # [512] -> [128, 4]
        return np.ascontiguousarray(np.asarray(b, np.float32).reshape(4, 128).T)

    b_c0 = colchunks(inputs["conv0_b"])
    b_c1 = colchunks(inputs["conv1_b"])
    bhw = np.empty((128, 4, 8), np.float32)
    for bl, (blk, lay) in enumerate((("hw0_b", 0), ("hw0_b", 1), ("hw1_b", 0), ("hw1_b", 1))):
        b = np.asarray(inputs[blk], np.float32)[lay]      # [1024]
        bhw[:, bl, 0:4] = b[:512].reshape(4, 128).T
        bhw[:, bl, 4:8] = b[512:1024].reshape(4, 128).T
    b_hw = np.ascontiguousarray(bhw.reshape(128, 32))
    b_pr = colchunks(inputs["proj_b"])

    # embedding table as lhsT row-chunks [128, 3*128]
    emb_lhs = np.zeros((128, 3 * 128), np.float32)
    emb_lhs[:, 0:128] = tok_emb[0:128]
    emb_lhs[:, 128:256] = tok_emb[128:256]
    emb_lhs[0:8, 256:384] = tok_emb[256:264]
    emb_lhs = bf(emb_lhs)
    emb_row4 = bf(tok_emb[BPE_MASK_IDX:BPE_MASK_IDX + 1, :])  # [1, 128]
    iota_c = np.empty((128, 3), np.float32)
    p = np.arange(128)
    for j in range(3):
        iota_c[:, j] = (j * 128 + p).astype(np.float32)

    shared = dict(emb_lhs=emb_lhs, emb_row4=emb_row4, iota_c=iota_c,
                  w_c0=w_c0, w_c1=w_c1, w_hw=w_hw, w_pr=w_pr,
                  b_c0=b_c0, b_c1=b_c1, b_hw=b_hw, b_pr=b_pr)

    in_maps = []
    meta = []
    for core in range(N_CORES):
        m = dict(shared)
        tok = np.empty((SEQ_PER_CORE, 128, T), _F16_NP)
        bpe = np.empty((SEQ_PER_CORE, 1, T), _BF16_NP)
        amsk = np.empty((SEQ_PER_CORE, 128, 2 * T), _BF16_NP)
        for s in range(SEQ_PER_CORE):
            b = core * SEQ_PER_CORE + s
            tok[s] = np.broadcast_to(byte_tokens[b].astype(_F16_NP), (128, T))
            bpe[s, 0] = (bpe_mask[b]).astype(_BF16_NP)
            pl = pool_lengths[b]
            cum = np.cumsum(pl)
            s_w = (cum - pl)
            a1 = np.full(T, NEG_BIG, np.float32)
            a2 = np.full(T, NEG_BIG, np.float32)
            st = s_w[pl > 1]
            a1[st[st < T]] = 0.0
            st = s_w[pl > 2]
            a2[st[st < T]] = 0.0
            amsk[s, :, 0:T] = np.broadcast_to(a1.astype(_BF16_NP), (128, T))
            amsk[s, :, T:2 * T] = np.broadcast_to(a2.astype(_BF16_NP), (128, T))
            meta.append((s_w, pl))
        m["tok_bc"] = tok
        m["bpe_row"] = bpe
        m["a_msk"] = amsk
        in_maps.append(m)
    return in_maps, meta


def kernel(**inputs) -> np.ndarray:
    from concourse.bass_utils import run_bass_kernel_spmd

    if "nc" not in _CACHE:
        _CACHE["nc"] = _build_program()
    nc = _CACHE["nc"]

    in_maps, meta = _prep_inputs(inputs)
    res = run_bass_kernel_spmd(nc, in_maps, list(range(N_CORES)))

    proj_b = np.asarray(inputs["proj_b"], np.float32)
    full = np.empty((BSZ, NW, WED), np.float32)
    for core in range(N_CORES):
        o = np.asarray(res.results[core]["out"], np.float32)  # [2, 512, T]
        for s in range(SEQ_PER_CORE):
            b = core * SEQ_PER_CORE + s
            s_w, pl = meta[b]
            cols = np.clip(s_w, 0, T - 1)
            full[b] = o[s][:, cols].T
            if (pl == 0).any():
                full[b][pl == 0] = proj_b
    return full


# revision 12
# speedup vs baseline: 1.2135x; 1.2135x over previous
"""Trainium2 Bass kernel for nn_ByteSequenceEmbedder.

Data-parallel across 8 NeuronCores: 2 sequences per core, weights replicated.

Per-core dataflow (all activations channels-on-partitions, "layout A" [C, T]):
  embed   : one-hot matmul — tokens broadcast [128,T] (host), DVE is_equal vs
            per-partition iota -> onehot chunks; PE: tok_emb-chunks.T @ onehot
            accumulated in PSUM (+ K=1 matmul adding bpe-marker row)
  conv0   : 3 shifted matmuls per (T-chunk, co-chunk) accumulating in PSUM,
            ReLU+bias fused into the ACT PSUM->SBUF evacuation
  highway : 2 blocks x 2 layers; 8x4 matmuls per T-chunk, ReLU/Sigmoid evac,
            DVE combine x' = g*(relu(h)-x)+x
  conv1   : 12 matmuls per (T-chunk, co-chunk) + residual add
  pool    : ragged word max-pool as masked shifted max:
            msel[t] = max(x2[t], x2[t+1]+A1[t], x2[t+2]+A2[t]) with host-built
            additive masks (0 where word@t has len>j, else -1e30)
  proj    : projection applied over ALL T positions; host selects column s_w
            per word while unsharding (empty pools -> proj_b row)

Matmul operands are bf16 (f32 PSUM accumulation).
"""
import numpy as np

import concourse.bacc as bacc
import concourse.tile as tile
import concourse.mybir as mybir

BSZ, NW, T = 16, 1024, 3072
BED, WED = 128, 512
VOCAB = 264
BPE_MASK_IDX = 4
N_CORES = 8
SEQ_PER_CORE = BSZ // N_CORES
TP = T + 2          # conv buffers: one zero halo col each side
TP2 = T + 4         # pooling source: 1 left + 3 right halo cols
NCH = T // 512      # T-chunks of 512
BF16 = mybir.dt.bfloat16
F16 = mybir.dt.float16
F32 = mybir.dt.float32

_BF16_NP = mybir.dt.np(BF16)
_F16_NP = np.float16
NEG_BIG = -1e30

_CACHE = {}


def _build_program():
    nc = bacc.Bacc("TRN2", target_bir_lowering=False, debug=False)

    def dram_in(name, shape, dt):
        return nc.dram_tensor(name, shape, dt, kind="ExternalInput").ap()

    emb_lhs = dram_in("emb_lhs", [128, 3 * 128], BF16)   # tok_emb row-chunks
    emb_row4 = dram_in("emb_row4", [1, 128], BF16)       # tok_emb[4]
    iota_c = dram_in("iota_c", [128, 3], F32)            # per-partition vocab iota
    w_c0 = dram_in("w_c0", [128, 3 * WED], BF16)         # [ci, k*512+co]
    w_c1 = dram_in("w_c1", [128, 4 * 3 * WED], BF16)     # [ci%128, (q*3+k)*512+co]
    w_hw = dram_in("w_hw", [128, 4 * 4 * 1024], BF16)    # [(bl*4+q)*1024 + co_out]
    w_pr = dram_in("w_pr", [128, 4 * WED], BF16)         # [q*512+co]
    b_c0 = dram_in("b_c0", [128, 4], F32)
    b_c1 = dram_in("b_c1", [128, 4], F32)
    b_hw = dram_in("b_hw", [128, 4 * 8], F32)            # [bl*8 + m]
    b_pr = dram_in("b_pr", [128, 4], F32)
    tok_bc = dram_in("tok_bc", [SEQ_PER_CORE, 128, T], F16)   # tokens bcast over partitions
    bpe_row = dram_in("bpe_row", [SEQ_PER_CORE, 1, T], BF16)  # bpe mask 0/1
    a_msk = dram_in("a_msk", [SEQ_PER_CORE, 128, 2 * T], BF16)  # pooling additive masks

    out = nc.dram_tensor("out", [SEQ_PER_CORE, WED, T], F32, kind="ExternalOutput").ap()

    RELU = mybir.ActivationFunctionType.Relu
    SIGM = mybir.ActivationFunctionType.Sigmoid
    IDEN = mybir.ActivationFunctionType.Identity
    MAX = mybir.AluOpType.max
    ADD = mybir.AluOpType.add
    SUB = mybir.AluOpType.subtract
    MUL = mybir.AluOpType.mult
    ISEQ = mybir.AluOpType.is_equal

    with tile.TileContext(nc) as tc:
        with tc.tile_pool(name="wp", bufs=1) as wp, \
             tc.tile_pool(name="ap", bufs=1) as apool, \
             tc.tile_pool(name="tp", bufs=3) as tp, \
             tc.tile_pool(name="pp", bufs=8, space="PSUM") as pp:

            # ---- HAM warm-up: PE activity from t~0 so real matmuls start at 2.4GHz ----
            wu = wp.tile([128, 512], BF16)
            nc.vector.memset(wu[:], 0)
            for _ in range(20):
                wps = pp.tile([128, 512], F32, tag="ps", name="wps")
                nc.tensor.matmul(out=wps[:], lhsT=wu[:, 0:128], rhs=wu[:],
                                 start=True, stop=True)

            # ---- load weights/biases once (sync queue; small/early first) ----
            t_embA = wp.tile([128, 3 * 128], BF16)
            t_row4 = wp.tile([1, 128], BF16)
            t_iota = wp.tile([128, 3], F32)
            t_bc0 = wp.tile([128, 4], F32)
            t_bc1 = wp.tile([128, 4], F32)
            t_bhw = wp.tile([128, 4 * 8], F32)
            t_bpr = wp.tile([128, 4], F32)
            t_wc0 = wp.tile([128, 3 * WED], BF16)
            t_wc1 = wp.tile([128, 4 * 3 * WED], BF16)
            t_whw = wp.tile([128, 4 * 4 * 1024], BF16)
            t_wpr = wp.tile([128, 4 * WED], BF16)
            # first token chunk for seq 0 ahead of the weight loads (critical path
            # to the very first real matmul); rest of seq-0 embed inputs follow on
            # the scalar queue inside embed_seq.
            t_tok0 = apool.tile([128, T], F16, tag="tok", name="t_tok0", bufs=2)
            nc.sync.dma_start(out=t_tok0[:, 0:512], in_=tok_bc[0, :, 0:512])
            for t, d in ((t_embA, emb_lhs), (t_row4, emb_row4), (t_iota, iota_c),
                         (t_bc0, b_c0), (t_bc1, b_c1), (t_bhw, b_hw), (t_bpr, b_pr),
                         (t_wc0, w_c0), (t_wc1, w_c1), (t_whw, w_hw), (t_wpr, w_pr)):
                nc.sync.dma_start(out=t[:], in_=d[:])

            def conv_block(X, Y, wt, bt, n_ci):
                """Y[:, chunk m cols 1..T] = relu(conv(X) + b)."""
                for n in range(NCH):
                    for m in range(4):
                        ps = pp.tile([128, 512], F32, tag="ps", name="ps")
                        nmm = n_ci * 3
                        i = 0
                        for q in range(n_ci):
                            for k in range(3):
                                lhs = wt[:, (q * 3 + k) * WED + m * 128:(q * 3 + k) * WED + (m + 1) * 128]
                                nc.tensor.matmul(
                                    out=ps[:], lhsT=lhs,
                                    rhs=X[:, q * TP + n * 512 + k:q * TP + n * 512 + k + 512],
                                    start=(i == 0), stop=(i == nmm - 1))
                                i += 1
                        dst = Y[:, m * TP + 1 + n * 512:m * TP + 1 + (n + 1) * 512]
                        nc.scalar.activation(out=dst, in_=ps[:], func=RELU,
                                             bias=bt[:, m:m + 1], scale=1.0)

            def highway_layer(X, Y, bl, ytp=TP):
                """Y = g*relu(h) + (1-g)*X; X [128, 4*TP], Y [128, 4*ytp]."""
                for n in range(NCH):
                    pss = []
                    for m in range(8):
                        ps = pp.tile([128, 512], F32, tag="ps", name="ps")
                        for q in range(4):
                            base = (bl * 4 + q) * 1024 + m * 128
                            nc.tensor.matmul(
                                out=ps[:], lhsT=t_whw[:, base:base + 128],
                                rhs=X[:, q * TP + 1 + n * 512:q * TP + 1 + (n + 1) * 512],
                                start=(q == 0), stop=(q == 3))
                        pss.append(ps)
                    for c in range(4):
                        xs = X[:, c * TP + 1 + n * 512:c * TP + 1 + (n + 1) * 512]
                        h_t = tp.tile([128, 512], BF16, tag="h", name="h_t")
                        g_t = tp.tile([128, 512], BF16, tag="g", name="g_t")
                        d_t = tp.tile([128, 512], BF16, tag="d", name="d_t")
                        nc.scalar.activation(out=h_t[:], in_=pss[c][:], func=RELU,
                                             bias=t_bhw[:, bl * 8 + c:bl * 8 + c + 1], scale=1.0)
                        nc.scalar.activation(out=g_t[:], in_=pss[4 + c][:], func=SIGM,
                                             bias=t_bhw[:, bl * 8 + 4 + c:bl * 8 + 4 + c + 1], scale=1.0)
                        nc.vector.tensor_tensor(out=d_t[:], in0=h_t[:], in1=xs, op=SUB)
                        nc.vector.tensor_tensor(out=d_t[:], in0=d_t[:], in1=g_t[:], op=MUL)
                        ys = Y[:, c * ytp + 1 + n * 512:c * ytp + 1 + (n + 1) * 512]
                        nc.vector.tensor_tensor(out=ys, in0=d_t[:], in1=xs, op=ADD)

            scope = nc.named_scope

            def embed_seq(s, t_tok=None):
                """One-hot-matmul embedding for sequence s -> x0 [128, TP] bf16.
                Token DMA is chunked so the first compare starts early."""
                ctx = scope(f"s{s}_embed"); ctx.__enter__()
                skip0 = t_tok is not None
                if t_tok is None:
                    t_tok = apool.tile([128, T], F16, tag="tok", name="t_tok", bufs=2)
                t_bpe = apool.tile([1, T], BF16, tag="bpe", name="t_bpe", bufs=2)
                t_am = apool.tile([128, 2 * T], BF16, tag="am", name="t_am")
                for n in range(1 if skip0 else 0, NCH):
                    nc.scalar.dma_start(out=t_tok[:, n * 512:(n + 1) * 512],
                                        in_=tok_bc[s, :, n * 512:(n + 1) * 512])
                nc.scalar.dma_start(out=t_bpe[:], in_=bpe_row[s])
                nc.scalar.dma_start(out=t_am[:], in_=a_msk[s])

                x0 = apool.tile([128, TP], BF16, tag="x0", name="x0", bufs=2)
                nc.vector.memset(x0[:, 0:1], 0)
                nc.vector.memset(x0[:, TP - 1:TP], 0)
                for n in range(NCH):
                    oh1 = tp.tile([128, 512], BF16, tag="oh1", name="oh1")
                    oh2 = tp.tile([128, 512], BF16, tag="oh2", name="oh2")
                    oh3 = tp.tile([8, 512], BF16, tag="oh3", name="oh3")
                    tb = t_tok[:, n * 512:(n + 1) * 512]
                    nc.vector.tensor_scalar(out=oh1[:], in0=tb, scalar1=t_iota[:, 0:1],
                                            scalar2=None, op0=ISEQ)
                    nc.vector.tensor_scalar(out=oh2[:], in0=tb, scalar1=t_iota[:, 1:2],
                                            scalar2=None, op0=ISEQ)
                    nc.vector.tensor_scalar(out=oh3[:], in0=t_tok[0:8, n * 512:(n + 1) * 512],
                                            scalar1=t_iota[0:8, 2:3], scalar2=None, op0=ISEQ)
                    ps = pp.tile([128, 512], F32, tag="ps", name="ps")
                    nc.tensor.matmul(out=ps[:], lhsT=t_embA[:, 0:128], rhs=oh1[:],
                                     start=True, stop=False)
                    nc.tensor.matmul(out=ps[:], lhsT=t_embA[:, 128:256], rhs=oh2[:],
                                     start=False, stop=False)
                    nc.tensor.matmul(out=ps[:], lhsT=t_embA[0:8, 256:384], rhs=oh3[:],
                                     start=False, stop=False)
                    nc.tensor.matmul(out=ps[:], lhsT=t_row4[:], rhs=t_bpe[:, n * 512:(n + 1) * 512],
                                     start=False, stop=True)
                    nc.scalar.activation(out=x0[:, 1 + n * 512:1 + (n + 1) * 512],
                                         in_=ps[:], func=IDEN, bias=0.0, scale=1.0)
                ctx.__exit__(None, None, None)
                return x0, t_am

            x0, t_am = embed_seq(0, t_tok=t_tok0)
            x1 = None
            for s in range(SEQ_PER_CORE):

                def act_buf(tag, w=TP, extra_halo=0):
                    b = apool.tile([128, 4 * w], BF16, tag=tag, name=tag)
                    for q in range(4):
                        nc.vector.memset(b[:, q * w:q * w + 1], 0)
                        nc.vector.memset(b[:, q * w + 1 + T:(q + 1) * w], 0)
                    return b

                # ---------- conv0 + highway block 0 ----------
                if x1 is None:
                    with scope(f"s{s}_conv0"):
                        x1 = act_buf("actA")
                        conv_block(x0, x1, t_wc0, t_bc0, 1)
                with scope(f"s{s}_hw0l0"):
                    x1b = act_buf("actB")
                    highway_layer(x1, x1b, 0)
                with scope(f"s{s}_hw0l1"):
                    x1c = act_buf("actC")
                    highway_layer(x1b, x1c, 1)

                # ---------- conv1 (+res) + highway block 1 ----------
                ctx = scope(f"s{s}_conv1"); ctx.__enter__()
                x2p = act_buf("actA")
                for n in range(NCH):
                    for m in range(4):
                        ps = pp.tile([128, 512], F32, tag="ps", name="ps")
                        i = 0
                        for q in range(4):
                            for k in range(3):
                                lhs = t_wc1[:, (q * 3 + k) * WED + m * 128:(q * 3 + k) * WED + (m + 1) * 128]
                                nc.tensor.matmul(
                                    out=ps[:], lhsT=lhs,
                                    rhs=x1c[:, q * TP + n * 512 + k:q * TP + n * 512 + k + 512],
                                    start=(i == 0), stop=(i == 11))
                                i += 1
                        r_t = tp.tile([128, 512], BF16, tag="h", name="r_t")
                        nc.scalar.activation(out=r_t[:], in_=ps[:], func=RELU,
                                             bias=t_bc1[:, m:m + 1], scale=1.0)
                        xs = x1c[:, m * TP + 1 + n * 512:m * TP + 1 + (n + 1) * 512]
                        nc.vector.tensor_tensor(
                            out=x2p[:, m * TP + 1 + n * 512:m * TP + 1 + (n + 1) * 512],
                            in0=r_t[:], in1=xs, op=ADD)
                ctx.__exit__(None, None, None)

                with scope(f"s{s}_hw1l0"):
                    x2b = act_buf("actB")
                    highway_layer(x2p, x2b, 2)
                with scope(f"s{s}_hw1l1"):
                    x2 = act_buf("actC", w=TP2)
                    highway_layer(x2b, x2, 3, ytp=TP2)

                # prefetch next sequence's embedding + conv0: fills the PE bubbles of
                # this sequence's DVE-bound pool phase
                if s + 1 < SEQ_PER_CORE:
                    next_x0, next_am = embed_seq(s + 1)
                    with scope(f"s{s + 1}_conv0"):
                        next_x1 = act_buf("actA")
                        conv_block(next_x0, next_x1, t_wc0, t_bc0, 1)

                # ---------- ragged max pool + projection, pipelined per T-chunk ----------
                ctx = scope(f"s{s}_poolproj"); ctx.__enter__()
                msel = apool.tile([128, 4 * T], BF16, tag="actB", name="msel")
                for n in range(NCH):
                    lo, hi = n * 512, (n + 1) * 512
                    for c in range(4):
                        base = c * TP2 + 1
                        s1 = tp.tile([128, 512], BF16, tag="s1", name="s1")
                        s2 = tp.tile([128, 512], BF16, tag="s2", name="s2")
                        nc.vector.tensor_tensor(out=s1[:], in0=x2[:, base + 1 + lo:base + 1 + hi],
                                                in1=t_am[:, lo:hi], op=ADD)
                        nc.vector.tensor_tensor(out=s2[:], in0=x2[:, base + 2 + lo:base + 2 + hi],
                                                in1=t_am[:, T + lo:T + hi], op=ADD)
                        nc.vector.tensor_tensor(out=s1[:], in0=s1[:], in1=s2[:], op=MAX)
                        nc.vector.tensor_tensor(out=msel[:, c * T + lo:c * T + hi],
                                                in0=s1[:], in1=x2[:, base + lo:base + hi], op=MAX)
                    for m in range(4):
                        ps = pp.tile([128, 512], F32, tag="ps", name="ps")
                        for q in range(4):
                            nc.tensor.matmul(
                                out=ps[:], lhsT=t_wpr[:, q * WED + m * 128:q * WED + (m + 1) * 128],
                                rhs=msel[:, q * T + lo:q * T + hi],
                                start=(q == 0), stop=(q == 3))
                        o_t = tp.tile([128, 512], F32, tag="o", name="o_t", bufs=4)
                        nc.scalar.activation(out=o_t[:], in_=ps[:], func=IDEN,
                                             bias=t_bpr[:, m:m + 1], scale=1.0)
                        nc.sync.dma_start(out=out[s, m * 128:(m + 1) * 128, lo:hi], in_=o_t[:])
                ctx.__exit__(None, None, None)
                if s + 1 < SEQ_PER_CORE:
                    x0, t_am, x1 = next_x0, next_am, next_x1
                else:
                    x1 = None

    nc.compile()
    return nc


def _prep_inputs(inputs):
    """Host-side: shard + convert to the kernel's DRAM tensor layouts."""
    byte_tokens = np.asarray(inputs["byte_tokens"], np.int64)
    bpe_mask = np.asarray(inputs["bpe_mask"], bool)
    pool_lengths = np.asarray(inputs["pool_lengths"], np.int64)
    tok_emb = np.asarray(inputs["tok_emb"], np.float32)

    def bf(x):
        return np.ascontiguousarray(np.asarray(x, np.float32).astype(_BF16_NP))

    conv0_W = np.asarray(inputs["conv0_W"], np.float32)   # [3,128,512]
    conv1_W = np.asarray(inputs["conv1_W"], np.float32)   # [3,512,512]
    hw0_W = np.asarray(inputs["hw0_W"], np.float32)       # [2,1024,512]
    hw1_W = np.asarray(inputs["hw1_W"], np.float32)
    proj_W = np.asarray(inputs["proj_W"], np.float32)     # [512,512]

    w_c0 = bf(conv0_W.transpose(1, 0, 2).reshape(128, 3 * WED))
    w_c1 = bf(conv1_W.transpose(1, 0, 2).reshape(4, 128, 3, WED)
              .transpose(1, 0, 2, 3).reshape(128, 4 * 3 * WED))
    whw = np.empty((128, 16, 1024), np.float32)
    for bl, (blk, lay) in enumerate(((hw0_W, 0), (hw0_W, 1), (hw1_W, 0), (hw1_W, 1))):
        wt = blk[lay].T  # [512, 1024]
        for q in range(4):
            whw[:, bl * 4 + q, :] = wt[q * 128:(q + 1) * 128]
    w_hw = bf(whw.reshape(128, 16 * 1024))
    w_pr = bf(proj_W.T.reshape(4, 128, WED).transpose(1, 0, 2).reshape(128, 4 * WED))

    def colchunks(b):  # [512] -> [128, 4]
        return np.ascontiguousarray(np.asarray(b, np.float32).reshape(4, 128).T)

    b_c0 = colchunks(inputs["conv0_b"])
    b_c1 = colchunks(inputs["conv1_b"])
    bhw = np.empty((128, 4, 8), np.float32)
    for bl, (blk, lay) in enumerate((("hw0_b", 0), ("hw0_b", 1), ("hw1_b", 0), ("hw1_b", 1))):
        b = np.asarray(inputs[blk], np.float32)[lay]      # [1024]
        bhw[:, bl, 0:4] = b[:512].reshape(4, 128).T
        bhw[:, bl, 4:8] = b[512:1024].reshape(4, 128).T
    b_hw = np.ascontiguousarray(bhw.reshape(128, 32))
    b_pr = colchunks(inputs["proj_b"])

    # embedding table as lhsT row-chunks [128, 3*128]
    emb_lhs = np.zeros((128, 3 * 128), np.float32)
    emb_lhs[:, 0:128] = tok_emb[0:128]
    emb_lhs[:, 128:256] = tok_emb[128:256]
    emb_lhs[0:8, 256:384] = tok_emb[256:264]
    emb_lhs = bf(emb_lhs)
    emb_row4 = bf(tok_emb[BPE_MASK_IDX:BPE_MASK_IDX + 1, :])  # [1, 128]
    iota_c = np.empty((128, 3), np.float32)
    p = np.arange(128)
    for j in range(3):
        iota_c[:, j] = (j * 128 + p).astype(np.float32)

    shared = dict(emb_lhs=emb_lhs, emb_row4=emb_row4, iota_c=iota_c,
                  w_c0=w_c0, w_c1=w_c1, w_hw=w_hw, w_pr=w_pr,
                  b_c0=b_c0, b_c1=b_c1, b_hw=b_hw, b_pr=b_pr)

    in_maps = []
    meta = []
    for core in range(N_CORES):
        m = dict(shared)
        tok = np.empty((SEQ_PER_CORE, 128, T), _F16_NP)
        bpe = np.empty((SEQ_PER_CORE, 1, T), _BF16_NP)
        amsk = np.empty((SEQ_PER_CORE, 128, 2 * T), _BF16_NP)
        for s in range(SEQ_PER_CORE):
            b = core * SEQ_PER_CORE + s
            tok[s] = np.broadcast_to(byte_tokens[b].astype(_F16_NP), (128, T))
            bpe[s, 0] = (bpe_mask[b]).astype(_BF16_NP)
            pl = pool_lengths[b]
            cum = np.cumsum(pl)
            s_w = (cum - pl)
            a1 = np.full(T, NEG_BIG, np.float32)
            a2 = np.full(T, NEG_BIG, np.float32)
            st = s_w[pl > 1]
            a1[st[st < T]] = 0.0
            st = s_w[pl > 2]
            a2[st[st < T]] = 0.0
            amsk[s, :, 0:T] = np.broadcast_to(a1.astype(_BF16_NP), (128, T))
            amsk[s, :, T:2 * T] = np.broadcast_to(a2.astype(_BF16_NP), (128, T))
            meta.append((s_w, pl))
        m["tok_bc"] = tok
        m["bpe_row"] = bpe
        m["a_msk"] = amsk
        in_maps.append(m)
    return in_maps, meta


def kernel(**inputs) -> np.ndarray:
    from concourse.bass_utils import run_bass_kernel_spmd

    if "nc" not in _CACHE:
        _CACHE["nc"] = _build_program()
    nc = _CACHE["nc"]

    in_maps, meta = _prep_inputs(inputs)
    res = run_bass_kernel_spmd(nc, in_maps, list(range(N_CORES)))

    proj_b = np.asarray(inputs["proj_b"], np.float32)
    full = np.empty((BSZ, NW, WED), np.float32)
    for core in range(N_CORES):
        o = np.asarray(res.results[core]["out"], np.float32)  # [2, 512, T]
        for s in range(SEQ_PER_CORE):
            b = core * SEQ_PER_CORE + s
            s_w, pl = meta[b]
            cols = np.clip(s_w, 0, T - 1)
            full[b] = o[s][:, cols].T
            if (pl == 0).any():
                full[b][pl == 0] = proj_b
    return full
